# revision 1
# baseline (speedup 1.0000x reference)
"""Conv1d (K=5, pad=2) with folded LoRA on 8 Trainium2 NeuronCores.

Strategy
--------
Data-parallel: batch 8 -> 1 batch item per core. The LoRA path is folded
into the conv weights on the host (exact up to fp32 rounding):
    W_eff = conv_w + (alpha/rank) * einsum('or,rik->oik', lora_B, lora_A)
so the device kernel is a single conv1d + bias.

Per core: y[co, t] = bias[co] + sum_{k,ci} W_eff[co, ci, k] * x[ci, t+k-2]
computed as 5 shifted matmuls accumulating in PSUM, over 2 ci-blocks and
2 co-blocks of 128, in fp32r (TF32-class PE mode, 1 cycle/row; ~1.5e-4
scale-relative absmax at K=128 contraction, measured on HW).

Toolchain constraint baked into the structure: every instruction may carry
at most ONE sync wait (walrus setupSyncWait limit), and Tile's wait elision
is per-proc (engine vs sequencer are distinct procs, no transitivity).
Hence:
  - PE "observer" matmuls (1-column, scratch PSUM) absorb each x/weight DMA
    lane wait so real matmuls only wait on the DVE sem (PSUM-bank WAR).
  - Evictions (PSUM->SBUF + bias add) run exclusively on DVE and wait only
    on PE; out-DMA-slot WAR is absorbed by tiny DVE memsets; the bias lane
    by a tiny DVE copy.
  - x-loads ride the SP HWDGE ring, stores the ACT HWDGE ring; same-ring
    WAW lane waits are absorbed by sequencer nops on the matching ring.
  - A tail chain of 1-dep sync nops covers all procs so the exit drain
    carries at most one wait.
"""
import sys
sys.path.insert(0, "/opt/trn_rl_repo")
import numpy as np

from concourse import bass, mybir, tile
from concourse import bass_utils
from concourse.tile import add_dep_helper

# Problem constants (hardcoded per contract)
B = 8
CI = 256
CO = 256
K = 5
PAD = 2
T = 16384
RANK = 8
ALPHA = 16.0
SCALING = ALPHA / RANK
N_CORES = 8

# Tiling
CHUNK = 1024          # output columns per chunk
NCHUNK = T // CHUNK   # 16
SUB = 512             # matmul free dim
NSUB = CHUNK // SUB   # 2
XCOLS = CHUNK + 2 * PAD  # chunk + halo


def _build_nc(reps=1):
    f32 = mybir.dt.float32
    f32r = mybir.dt.float32r

    nc = bass.Bass(trn_type="TRN2", debug=False)
    x = nc.dram_tensor("x", [CI, T], f32, kind="ExternalInput").ap()
    wts = nc.dram_tensor("wts", [128, K * 2 * 2 * 128], f32, kind="ExternalInput").ap()
    bias = nc.dram_tensor("bias", [128, 2], f32, kind="ExternalInput").ap()
    zeros = nc.dram_tensor("zeros", [128, 2, PAD], f32, kind="ExternalInput").ap()
    # one output tensor per 2048-wide super-chunk, stored via SWDGE so each
    # store owns a DMASW lane exactly once (no lane-predecessor wait); host
    # concatenates
    ys = [nc.dram_tensor(f"y{s}", [CI, 2 * CHUNK], f32, kind="ExternalOutput").ap()
          for s in range(NCHUNK // 2)]

    xab = x.rearrange("(b p) t -> p b t", p=128)
    ysab = [yc.rearrange("(b p) t -> p b t", p=128) for yc in ys]

    NOB = 2   # out staging buffers (super-chunks)
    NPB = 6   # psum accumulation banks

    with tile.TileContext(nc) as tc:
        with tc.tile_pool(name="wp", bufs=1) as wp, \
             tc.tile_pool(name="pp", bufs=1, space="PSUM") as pp:

            # write-once observer scratch: two columns per observer matmul
            # (fp32r APs need 8-byte alignment)
            obs_ps = pp.tile([128, 64], f32, name="obs_ps", tag="obs")
            pbufs = [pp.tile([128, SUB], f32, name=f"pt{j}", tag=f"pt{j}")
                     for j in range(NPB)]
            # x is fully resident: one dedicated buffer per chunk, no reuse
            xbufs = [wp.tile([128, 2, XCOLS], f32r, name=f"xt{j}", tag=f"xt{j}")
                     for j in range(NCHUNK)]
            obufs = [wp.tile([128, 2, 2 * CHUNK], f32, name=f"ot{j}", tag=f"ot{j}")
                     for j in range(NOB)]
            # write-once DVE gate scratch: one column per gate memset
            gs = wp.tile([128, 4 * NCHUNK * reps + 8], f32, name="gs")

            wr = wp.tile([128, K * 2 * 2 * 128], f32r, name="wr")
            d_w = nc.sync.dma_start(out=wr[:], in_=wts[:].bitcast(f32r))
            bs = wp.tile([128, 2], f32, name="bs")
            d_b = nc.sync.dma_start(out=bs[:], in_=bias[:])

            n_obs = [0]

            def pe_observe(src_ap, dma_inst):
                """1-column matmul whose only wait is `dma_inst`'s lane.

                Reads only within the region `dma_inst` wrote; writes its own
                never-reused obs_ps column (no WAW chain)."""
                n = src_ap.shape[-1]
                m = min(128, n)
                oc = 2 * n_obs[0]
                n_obs[0] += 1
                mm = nc.tensor.matmul(obs_ps[0:m, oc:oc + 2], src_ap[:, 0:m],
                                      src_ap[:, 0:2], start=True, stop=True)
                add_dep_helper(mm.ins, dma_inst.ins, sync=False, reason="obs-order")
                return mm

            n_gate = [0]

            def dve_gate(dep_inst):
                """Write-once DVE memset whose only wait is dep's proc tick."""
                gc = n_gate[0]
                n_gate[0] += 1
                ms = nc.vector.memset(gs[:, gc:gc + 1], 0.0)
                add_dep_helper(ms.ins, dep_inst.ins, sync=True, reason="dve-gate")
                return ms

            obs_w = pe_observe(wr, d_w)
            # DVE observes the bias lane via a write-once copy
            bscratch = wp.tile([128, 2], f32, name="bscratch")
            obs_b = nc.vector.tensor_copy(bscratch[:], bs[:])

            in_dmas = []      # list of lists per chunk
            out_dmas = []     # per super-chunk (final rep only)
            sc_evicts = {}    # global super-chunk -> last evict
            sc_ods = {}       # global super-chunk -> out dma
            last_mm = None
            last_evict = None
            pi = 0            # psum bank rotation
            NSC = NCHUNK // 2

            for r in range(reps):
                for c in range(NCHUNK):
                    lo = c * CHUNK - PAD
                    xt = xbufs[c]

                    observers = []
                    if r == 0:
                        chunk_dmas = []
                        if c == 0:
                            chunk_dmas.append(nc.sync.dma_start(
                                out=xt[:, :, PAD:XCOLS],
                                in_=xab[:, :, 0:CHUNK + PAD].bitcast(f32r)))
                            chunk_dmas.append(nc.sync.dma_start(
                                out=xt[:, :, 0:PAD], in_=zeros[:].bitcast(f32r)))
                        elif c == NCHUNK - 1:
                            chunk_dmas.append(nc.sync.dma_start(
                                out=xt[:, :, 0:CHUNK + PAD],
                                in_=xab[:, :, lo:T].bitcast(f32r)))
                            chunk_dmas.append(nc.sync.dma_start(
                                out=xt[:, :, CHUNK + PAD:XCOLS],
                                in_=zeros[:].bitcast(f32r)))
                        else:
                            chunk_dmas.append(nc.sync.dma_start(
                                out=xt[:], in_=xab[:, :, lo:lo + XCOLS].bitcast(f32r)))
                        in_dmas.append(chunk_dmas)

                        # PE observes this chunk's x lanes via 1-col matmuls;
                        # each observer reads only within its DMA's region.
                        for i, d in enumerate(chunk_dmas):
                            if i == 0:
                                src_ap = (xt[:, 0, PAD:PAD + 128] if c == 0
                                          else xt[:, 0, 0:128])
                            else:
                                src_ap = (xt[:, 0, 0:PAD] if c == 0
                                          else xt[:, 0, CHUNK + PAD:XCOLS])
                            observers.append(pe_observe(src_ap, d))

                    sc, half = divmod(c, 2)
                    gsc = r * NSC + sc
                    ot = obufs[gsc % NOB]
                    evict_gates = [obs_b]
                    if half == 0 and gsc >= NOB:
                        # pre-lift the recycled out buffer's history onto
                        # DVE's observed clock: one 1-wait gate per proc
                        old = gsc - NOB
                        evict_gates.append(dve_gate(sc_evicts[old]))
                        if old in sc_ods:
                            evict_gates.append(dve_gate(sc_ods[old]))

                    first_evict_of_chunk = True
                    for co in range(2):
                        for ts in range(NSUB):
                            pt = pbufs[pi % NPB]
                            pi += 1
                            first = True
                            for b in range(2):
                                for k in range(K):
                                    widx = ((k * 2 + b) * 2 + co) * 128
                                    mm = nc.tensor.matmul(
                                        pt[:],
                                        wr[:, widx:widx + 128],
                                        xt[:, b, ts * SUB + k: ts * SUB + k + SUB],
                                        start=first,
                                        stop=(b == 1 and k == K - 1),
                                    )
                                    if first:
                                        for ob in observers:
                                            add_dep_helper(
                                                mm.ins, ob.ins, sync=False,
                                                reason="order-after-observe")
                                    first = False
                                    last_mm = mm
                            off = half * CHUNK + ts * SUB
                            ev = nc.vector.tensor_scalar_add(
                                out=ot[:, co, off:off + SUB],
                                in0=pt[:],
                                scalar1=bs[:, co:co + 1],
                            )
                            if first_evict_of_chunk:
                                for g in evict_gates:
                                    add_dep_helper(ev.ins, g.ins, sync=False,
                                                   reason="order-after-gate")
                                first_evict_of_chunk = False
                            last_evict = ev

                    if half == 1:
                        sc_evicts[gsc] = last_evict
                        if r == reps - 1:
                            # SWDGE store: own output tensor + own DMASW lane
                            od = nc.gpsimd.dma_start(out=ysab[sc][:], in_=ot[:])
                            sc_ods[gsc] = od
                            out_dmas.append(od)

            # Tail flush: cover every proc with 1-dep sync nops so the final
            # drain carries at most one wait.
            tail_deps = [d for ds in in_dmas[-8:] for d in ds] + out_dmas + \
                [last_mm, last_evict]
            for dep in tail_deps:
                nop = nc.sync.nop()
                add_dep_helper(nop.ins, dep.ins, sync=True, reason="tailflush")

    return nc


def check_waits(nc):
    """Return instructions carrying more than one sync wait (walrus limit)."""
    bad = []
    for f in nc.m.functions:
        for bb in f.blocks:
            for inst in bb.instructions:
                si = inst.sync_info
                nw = len(si.on_wait) if si and si.on_wait else 0
                if nw > 1:
                    bad.append((inst.name, type(inst).__name__, nw,
                                [w.ant_name for w in si.on_wait]))
    return bad


def _pack_weights(conv_w, conv_b, lora_A, lora_B):
    w_eff = conv_w.astype(np.float32) + (
        SCALING * np.einsum(
            "or,rik->oik", lora_B.astype(np.float64),
            lora_A.astype(np.float64).reshape(RANK, CI, K))
    ).astype(np.float32)
    # wts[ci_in_block, ((k*2 + b)*2 + co)*128 + m] = w_eff[co*128+m, b*128+ci, k]
    a = w_eff.reshape(2, 128, 2, 128, K)        # [co_blk, m, ci_blk, ci, k]
    a = a.transpose(3, 4, 2, 0, 1)              # [ci, k, b, co_blk, m]
    wts = np.ascontiguousarray(a.reshape(128, K * 2 * 2 * 128), dtype=np.float32)
    bias = np.ascontiguousarray(
        conv_b.astype(np.float32).reshape(2, 128).T)  # [128, 2]
    return wts, bias


_CACHED_NC = None


def kernel(x, conv_w, conv_b, lora_A, lora_B, _trace=False):
    global _CACHED_NC
    x = np.asarray(x, dtype=np.float32)
    wts, bias = _pack_weights(np.asarray(conv_w), np.asarray(conv_b),
                              np.asarray(lora_A), np.asarray(lora_B))
    zeros = np.zeros((128, 2, PAD), dtype=np.float32)

    if _CACHED_NC is None:
        _CACHED_NC = _build_nc()
        bad = check_waits(_CACHED_NC)
        assert not bad, f"sync-wait violations: {bad[:5]}"
    nc = _CACHED_NC

    in_maps = [
        {"x": x[i], "wts": wts, "bias": bias, "zeros": zeros}
        for i in range(N_CORES)
    ]
    res = bass_utils.run_bass_kernel_spmd(
        nc, in_maps, core_ids=list(range(N_CORES)), trace=_trace)
    out = np.stack(
        [np.concatenate([res.results[i][f"y{s}"] for s in range(NCHUNK // 2)],
                        axis=1)
         for i in range(N_CORES)], axis=0)
    if _trace:
        kernel._last_exec_time_ns = res.exec_time_ns
        kernel._last_results = res
    return out


if __name__ == "__main__":
    nc = _build_nc()
    bad = check_waits(nc)
    print("violations:", bad[:10])
    n_inst = sum(len(bb.instructions) for f in nc.m.functions for bb in f.blocks)
    print("instructions:", n_inst)



# revision 2
# speedup vs baseline: 1.4931x; 1.4931x over previous
"""Conv1d (K=5, pad=2) with folded LoRA on 8 Trainium2 NeuronCores.

Strategy
--------
Data-parallel: batch 8 -> 1 batch item per core. The LoRA path is folded
into the conv weights on the host (exact up to fp32 rounding):
    W_eff = conv_w + (alpha/rank) * einsum('or,rik->oik', lora_B, lora_A)
so the device kernel is a single conv1d + bias.

The conv runs in fp8-e4m3 DoubleRow matmuls (0.5 PE cycles per output
column at 256-deep contraction = both 128-ci blocks per tap via the
row-pair dim). Precision is recovered with a 3-term decomposition, all
host-quantized with the same power-of-2 scales (W x512, x x32) so every
pass accumulates into the same PSUM group:
    y ~ (wh + wl) @ xh + wh @ xl        [wl = fp8 resid of W, xl of x]
dropping only the wl@xl cross term (~6e-5 relative). Measured end-to-end
absmax-relative error ~2.5e-3 (fp8 quantization 1.2e-3 + PE fp8
accumulation path ~2e-3), vs the 2e-2 gate.

Per 512-col output chunk and co-block: 15 DoubleRow matmuls (5 taps x 3
terms) accumulate in one PSUM bank; DVE evicts with a single
tensor_scalar (psum * 2^-14 + bias) into SBUF staging; SWDGE stores
super-chunks sized (3,3,2,2,2,2,1,1)*1024 cols so the final drain tail
is a small store.

Toolchain constraint baked into the structure: every instruction may
carry at most ONE sync wait (walrus setupSyncWait limit), and Tile's
wait elision is per-proc. Hence:
  - PE "observer" matmuls (1-col, scratch PSUM) absorb each x/weight DMA
    lane wait so real matmuls only wait on the DVE sem (PSUM-bank WAR).
  - Evictions run exclusively on DVE and wait only on PE; out-staging
    WAR is absorbed by tiny DVE memsets; the bias lane by a DVE copy.
  - xh loads ride the SP HWDGE ring, xl loads the ACT HWDGE ring;
    stores go through SWDGE so each owns a DMASW lane exactly once.
  - A tail chain of 1-dep sync nops covers all procs so the exit drain
    carries at most one wait.
"""
import sys
sys.path.insert(0, "/opt/trn_rl_repo")
import numpy as np
import ml_dtypes

from concourse import bass, mybir, tile
from concourse import bass_utils
from concourse.tile import add_dep_helper

E4 = ml_dtypes.float8_e4m3

# Problem constants (hardcoded per contract)
B = 8
CI = 256
CO = 256
K = 5
PAD = 2
T = 16384
RANK = 8
ALPHA = 16.0
SCALING = ALPHA / RANK
N_CORES = 8

WS = 512.0            # weight quantization scale (power of 2)
XS = 32.0             # x quantization scale (power of 2)
OSCALE = 1.0 / (WS * XS)   # 2^-14, applied at eviction

# Tiling
CHUNK = 1024          # output columns per chunk
NCHUNK = T // CHUNK   # 16
SUB = 512             # matmul free dim (one PSUM bank)
NSUB = CHUNK // SUB   # 2
XCOLS = CHUNK + 2 * PAD  # chunk + halo (1028)
XAL = 1040            # allocated x-tile cols; pair-dim step must be %16==0
NSTAT = K * 2 * 2     # stationary tiles: (tap, hi/lo, co_blk)
# super-chunk sizes in chunks; 8 stores, one DMASW lane each; small tail
SC_SIZES = [3, 3, 2, 2, 2, 2, 1, 1]


def _build_nc():
    f32 = mybir.dt.float32
    f8 = mybir.dt.float8e4

    nc = bass.Bass(trn_type="TRN2", debug=False)
    xh = nc.dram_tensor("xh", [CI, T], f8, kind="ExternalInput").ap()
    xl = nc.dram_tensor("xl", [CI, T], f8, kind="ExternalInput").ap()
    wts = nc.dram_tensor("wts", [128, NSTAT * 2 * 128], f8,
                         kind="ExternalInput").ap()
    bias = nc.dram_tensor("bias", [128, 2], f32, kind="ExternalInput").ap()
    zeros = nc.dram_tensor("zeros", [128, 2, PAD], f8, kind="ExternalInput").ap()
    ys = [nc.dram_tensor(f"y{s}", [CI, SC_SIZES[s] * CHUNK], f32,
                         kind="ExternalOutput").ap()
          for s in range(len(SC_SIZES))]

    xhab = xh.rearrange("(b p) t -> p b t", p=128)
    xlab = xl.rearrange("(b p) t -> p b t", p=128)
    ysab = [yc.rearrange("(b p) t -> p b t", p=128) for yc in ys]

    sc_of_chunk = []          # chunk index -> (sc, base chunk of sc)
    base = 0
    for s, n in enumerate(SC_SIZES):
        for _ in range(n):
            sc_of_chunk.append((s, base))
        base += n

    NOB = 2   # out staging buffers (max super-chunk size each)
    NPB = 6   # psum accumulation banks
    MAXSC = max(SC_SIZES)

    with tile.TileContext(nc) as tc:
        with tc.tile_pool(name="wp", bufs=1) as wp, \
             tc.tile_pool(name="pp", bufs=1, space="PSUM") as pp:

            # write-once observer scratch: two columns per observer matmul
            obs_ps = pp.tile([128, 128], f32, name="obs_ps", tag="obs")
            pbufs = [pp.tile([128, SUB], f32, name=f"pt{j}", tag=f"pt{j}")
                     for j in range(NPB)]
            # x fully resident: one dedicated buffer per chunk, no reuse
            xhbufs = [wp.tile([128, 2, XAL], f8, name=f"xht{j}", tag=f"xht{j}")
                      for j in range(NCHUNK)]
            xlbufs = [wp.tile([128, 2, XAL], f8, name=f"xlt{j}", tag=f"xlt{j}")
                      for j in range(NCHUNK)]
            obufs = [wp.tile([128, 2, MAXSC * CHUNK], f32, name=f"ot{j}",
                             tag=f"ot{j}")
                     for j in range(NOB)]
            # write-once DVE gate scratch: one column per gate memset
            gs = wp.tile([128, 4 * NCHUNK + 8], f32, name="gs")

            wr = wp.tile([128, NSTAT, 2, 128], f8, name="wr")
            d_w = nc.sync.dma_start(
                out=wr[:],
                in_=wts.rearrange("p (s b m) -> p s b m", s=NSTAT, b=2))
            bs = wp.tile([128, 2], f32, name="bs")
            d_b = nc.sync.dma_start(out=bs[:], in_=bias[:])

            n_obs = [0]

            def pe_observe(src_ap, dma_inst):
                """1-column matmul whose only wait is `dma_inst`'s lane.

                Reads only within the region `dma_inst` wrote; writes its
                own never-reused obs_ps column (no WAW chain)."""
                n = src_ap.shape[-1]
                m = min(128, n)
                oc = 2 * n_obs[0]
                n_obs[0] += 1
                mm = nc.tensor.matmul(obs_ps[0:m, oc:oc + 2], src_ap[:, 0:m],
                                      src_ap[:, 0:2], start=True, stop=True)
                add_dep_helper(mm.ins, dma_inst.ins, sync=False,
                               reason="obs-order")
                return mm

            n_gate = [0]

            def dve_gate(dep_inst):
                """Write-once DVE memset whose only wait is dep's proc tick."""
                gc = n_gate[0]
                n_gate[0] += 1
                ms = nc.vector.memset(gs[:, gc:gc + 1], 0.0)
                add_dep_helper(ms.ins, dep_inst.ins, sync=True,
                               reason="dve-gate")
                return ms

            obs_w = pe_observe(wr[:, 0, 0], d_w)
            # DVE observes the bias lane via a write-once copy
            bscratch = wp.tile([128, 2], f32, name="bscratch")
            obs_b = nc.vector.tensor_copy(bscratch[:], bs[:])

            in_dmas = []      # list of lists per chunk
            out_dmas = []
            sc_evicts = {}    # super-chunk -> last evict
            sc_ods = {}       # super-chunk -> out dma
            last_mm = None
            last_evict = None
            pi = 0            # psum bank rotation

            for c in range(NCHUNK):
                lo = c * CHUNK - PAD
                chunk_dmas = []
                observers = []
                for xt, xab, eng in ((xhbufs[c], xhab, nc.sync),
                                     (xlbufs[c], xlab, nc.scalar)):
                    dmas = []
                    if c == 0:
                        dmas.append(eng.dma_start(
                            out=xt[:, :, PAD:XCOLS],
                            in_=xab[:, :, 0:CHUNK + PAD]))
                        dmas.append(eng.dma_start(
                            out=xt[:, :, 0:PAD], in_=zeros[:]))
                    elif c == NCHUNK - 1:
                        dmas.append(eng.dma_start(
                            out=xt[:, :, 0:CHUNK + PAD],
                            in_=xab[:, :, lo:T]))
                        dmas.append(eng.dma_start(
                            out=xt[:, :, CHUNK + PAD:XCOLS],
                            in_=zeros[:]))
                    else:
                        dmas.append(eng.dma_start(
                            out=xt[:, :, 0:XCOLS],
                            in_=xab[:, :, lo:lo + XCOLS]))
                    chunk_dmas.extend(dmas)
                    # PE observes this chunk's x lanes via 1-col matmuls;
                    # each observer reads only within its DMA's region.
                    for i, d in enumerate(dmas):
                        if i == 0:
                            src_ap = (xt[:, 0, PAD:PAD + 128] if c == 0
                                      else xt[:, 0, 0:128])
                        else:
                            src_ap = (xt[:, 0, 0:PAD] if c == 0
                                      else xt[:, 0, CHUNK + PAD:XCOLS])
                        observers.append(pe_observe(src_ap, d))
                in_dmas.append(chunk_dmas)

                sc, b0 = sc_of_chunk[c]
                ot = obufs[sc % NOB]
                evict_gates = [obs_b]
                if c == b0 and sc >= NOB:
                    # pre-lift the recycled out buffer's history onto
                    # DVE's observed clock: one 1-wait gate per proc
                    old = sc - NOB
                    evict_gates.append(dve_gate(sc_evicts[old]))
                    if old in sc_ods:
                        evict_gates.append(dve_gate(sc_ods[old]))

                first_evict_of_chunk = True
                xht, xlt = xhbufs[c], xlbufs[c]
                for co in range(2):
                    for ts in range(NSUB):
                        pt = pbufs[pi % NPB]
                        pi += 1
                        first = True
                        for k in range(K):
                            off = ts * SUB + k
                            sh = (k * 2 + 0) * 2 + co   # wh tile
                            sl = (k * 2 + 1) * 2 + co   # wl tile
                            for wtile, xtile in ((sh, xht), (sh, xlt),
                                                 (sl, xht)):
                                mm = nc.tensor.matmul(
                                    pt[:],
                                    wr[:, wtile],
                                    xtile[:, :, off:off + SUB],
                                    start=first,
                                    stop=(k == K - 1 and wtile == sl),
                                    perf_mode=mybir.MatmulPerfMode.DoubleRow,
                                )
                                if first:
                                    for ob in observers:
                                        add_dep_helper(
                                            mm.ins, ob.ins, sync=False,
                                            reason="order-after-observe")
                                first = False
                                last_mm = mm
                        ooff = (c - b0) * CHUNK + ts * SUB
                        ev = nc.vector.tensor_scalar(
                            out=ot[:, co, ooff:ooff + SUB],
                            in0=pt[:],
                            scalar1=float(OSCALE),
                            scalar2=bs[:, co:co + 1],
                            op0=mybir.AluOpType.mult,
                            op1=mybir.AluOpType.add,
                        )
                        if first_evict_of_chunk:
                            for g in evict_gates:
                                add_dep_helper(ev.ins, g.ins, sync=False,
                                               reason="order-after-gate")
                            first_evict_of_chunk = False
                        last_evict = ev

                if c - b0 == SC_SIZES[sc] - 1:
                    sc_evicts[sc] = last_evict
                    # SWDGE store: own output tensor + own DMASW lane
                    od = nc.gpsimd.dma_start(
                        out=ysab[sc][:],
                        in_=ot[:, :, 0:SC_SIZES[sc] * CHUNK])
                    sc_ods[sc] = od
                    out_dmas.append(od)

            # Tail flush: cover every proc with 1-dep sync nops so the
            # final drain carries at most one wait.
            tail_deps = [d for ds in in_dmas[-8:] for d in ds] + out_dmas + \
                [last_mm, last_evict]
            for dep in tail_deps:
                nop = nc.sync.nop()
                add_dep_helper(nop.ins, dep.ins, sync=True, reason="tailflush")

    return nc


def check_waits(nc):
    """Return instructions carrying more than one sync wait (walrus limit)."""
    bad = []
    for f in nc.m.functions:
        for bb in f.blocks:
            for inst in bb.instructions:
                si = inst.sync_info
                nw = len(si.on_wait) if si and si.on_wait else 0
                if nw > 1:
                    bad.append((inst.name, type(inst).__name__, nw,
                                [w.ant_name for w in si.on_wait]))
    return bad


def _q8(a):
    return np.asarray(a, dtype=np.float32).astype(E4)


def _pack_weights(conv_w, conv_b, lora_A, lora_B):
    w_eff = conv_w.astype(np.float32) + (
        SCALING * np.einsum(
            "or,rik->oik", lora_B.astype(np.float64),
            lora_A.astype(np.float64).reshape(RANK, CI, K))
    ).astype(np.float32)
    wp = w_eff * np.float32(WS)
    wh = _q8(wp)
    wl = _q8(wp - wh.astype(np.float32))
    # wts[ki, ((k*2+hl)*2+co_blk)*256 + b*128 + m]
    #   = w_hl[co_blk*128+m, b*128+ki, k] * WS
    parts = np.empty((128, K, 2, 2, 2, 128), dtype=E4)  # ki,k,hl,cob,b,m
    for hl, arr in ((0, wh), (1, wl)):
        a = arr.reshape(2, 128, 2, 128, K)      # [co_blk, m, b, ki, k]
        parts[:, :, hl] = a.transpose(3, 4, 0, 2, 1)  # -> [ki, k, cob, b, m]
    wts = np.ascontiguousarray(parts.reshape(128, NSTAT * 2 * 128))
    bias = np.ascontiguousarray(
        conv_b.astype(np.float32).reshape(2, 128).T)  # [128, 2]
    return wts, bias


_CACHED_NC = None


def kernel(x, conv_w, conv_b, lora_A, lora_B, _trace=False):
    global _CACHED_NC
    x = np.asarray(x, dtype=np.float32)
    wts, bias = _pack_weights(np.asarray(conv_w), np.asarray(conv_b),
                              np.asarray(lora_A), np.asarray(lora_B))
    xs = x * np.float32(XS)
    xh = _q8(xs)
    xlo = _q8(xs - xh.astype(np.float32))
    zeros = np.zeros((128, 2, PAD), dtype=E4)

    if _CACHED_NC is None:
        _CACHED_NC = _build_nc()
        bad = check_waits(_CACHED_NC)
        assert not bad, f"sync-wait violations: {bad[:5]}"
    nc = _CACHED_NC

    in_maps = [
        {"xh": xh[i], "xl": xlo[i], "wts": wts, "bias": bias, "zeros": zeros}
        for i in range(N_CORES)
    ]
    res = bass_utils.run_bass_kernel_spmd(
        nc, in_maps, core_ids=list(range(N_CORES)), trace=_trace)
    out = np.stack(
        [np.concatenate([res.results[i][f"y{s}"]
                         for s in range(len(SC_SIZES))], axis=1)
         for i in range(N_CORES)], axis=0)
    if _trace:
        kernel._last_exec_time_ns = res.exec_time_ns
        kernel._last_results = res
    return out


if __name__ == "__main__":
    nc = _build_nc()
    bad = check_waits(nc)
    print("violations:", bad[:10])
    n_inst = sum(len(bb.instructions) for f in nc.m.functions for bb in f.blocks)
    print("instructions:", n_inst)


# revision 20
# speedup vs baseline: 1.5875x; 1.0632x over previous
"""Conv1d (K=5, pad=2) with folded LoRA on 8 Trainium2 NeuronCores.

Strategy
--------
Data-parallel: batch 8 -> 1 batch item per core. The LoRA path is folded
into the conv weights on the host (exact up to fp32 rounding):
    W_eff = conv_w + (alpha/rank) * einsum('or,rik->oik', lora_B, lora_A)
so the device kernel is a single conv1d + bias.

The conv runs in fp8-e4m3 DoubleRow matmuls (0.5 PE cycles per output
column at 256-deep contraction = both 128-ci blocks per tap via the
row-pair dim). Precision is recovered with a 3-term decomposition, all
host-quantized with the same power-of-2 scales (W x512, x x32) so every
pass accumulates into the same PSUM group:
    y ~ (wh + wl) @ xh + wh @ xl        [wl = fp8 resid of W, xl of x]
dropping only the wl@xl cross term (~6e-5 relative). Measured end-to-end
absmax-relative error ~1.2e-3 vs the 2e-2 gate.

Per 512-col output group and co-block: 15 DoubleRow matmuls accumulate
in one PSUM bank in pass order (wh@xh x5, wh@xl x5, wl@xh x5); DVE
evicts with a single tensor_scalar (psum * 2^-14 + bias) into per-chunk
SBUF staging; each chunk is stored to DRAM by its own SWDGE DMA as soon
as its last eviction lands (streaming stores - the exit drain only waits
on the final 512-col store). SWDGE has 8 DMASW lanes; stores 9+ absorb
the lane-recycle wait with a Pool-engine gate memset.

Fill path: the first two and last two chunks are 512 columns so the
first matmul needs only a 516-col xh slice plus the 5-tap wh(co0)
weight quarter (weights are split into 4 DMAs: wh/wl x co-block); pad
columns at the t=0/t=T edges are DVE memsets, not DMAs. A chain of PE
"warmup" matmuls on a memset scratch tile keeps PE busy from ~1.5us so
the p-state ramp completes before the first real matmul.

Toolchain constraint baked into the structure: every instruction may
carry at most ONE sync wait (walrus setupSyncWait limit), and Tile's
wait elision is per-proc. Hence:
  - PE "observer" matmuls (1-col, scratch PSUM) absorb each x/weight DMA
    lane wait so real matmuls only wait on the DVE sem (PSUM-bank WAR).
    Observers are issued inline right before the first matmul that needs
    them (PE executes in order - an early observer stalls the stream).
  - Evictions run exclusively on DVE and wait only on PE; out-staging
    WAR is absorbed by tiny DVE memsets; the bias lane by a DVE copy.
  - xh loads ride the SP HWDGE ring, xl loads the ACT HWDGE ring;
    stores go through SWDGE (Pool).
  - A tail chain of 1-dep sync nops covers all procs so the exit drain
    carries at most one wait.
"""
import sys
sys.path.insert(0, "/opt/trn_rl_repo")
import numpy as np
import ml_dtypes

from concourse import bass, mybir, tile
from concourse import bass_utils
from concourse.tile import add_dep_helper

E4 = ml_dtypes.float8_e4m3

# Problem constants (hardcoded per contract)
B = 8
CI = 256
CO = 256
K = 5
PAD = 2
T = 16384
RANK = 8
ALPHA = 16.0
SCALING = ALPHA / RANK
N_CORES = 8

WS = 512.0            # weight quantization scale (power of 2)
XS = 32.0             # x quantization scale (power of 2)
OSCALE = 1.0 / (WS * XS)   # 2^-14, applied at eviction

SUB = 512             # matmul free dim (one PSUM bank)
# chunk sizes in output columns; 512-col edges for fast fill/drain
CHUNKS = [512, 512] + [1024] * 14 + [512, 512]
assert sum(CHUNKS) == T
NCHUNK = len(CHUNKS)
CSTART = [sum(CHUNKS[:i]) for i in range(NCHUNK)]
N_WARM = 5            # PE warmup matmuls (tuned against TimelineSim)


def _build_nc():
    f32 = mybir.dt.float32
    f8 = mybir.dt.float8e4

    nc = bass.Bass(trn_type="TRN2", debug=False)
    xh = nc.dram_tensor("xh", [CI, T], f8, kind="ExternalInput").ap()
    xl = nc.dram_tensor("xl", [CI, T], f8, kind="ExternalInput").ap()
    w0h = nc.dram_tensor("w0h", [128, K * 2 * 128], f8,
                         kind="ExternalInput").ap()
    w0l = nc.dram_tensor("w0l", [128, K * 2 * 128], f8,
                         kind="ExternalInput").ap()
    w1c = nc.dram_tensor("w1c", [128, 2 * K * 2 * 128], f8,
                         kind="ExternalInput").ap()
    xps = [nc.dram_tensor(f"x{c}p", [128, 2 * 2 * (CHUNKS[c] + 2 * PAD + 12)],
                          f8, kind="ExternalInput").ap()
           for c in range(4)]
    bias = nc.dram_tensor("bias", [128, 2], f32, kind="ExternalInput").ap()
    ys = [nc.dram_tensor(f"y{c}", [CI, CHUNKS[c]], mybir.dt.bfloat16,
                         kind="ExternalOutput").ap()
          for c in range(NCHUNK)]

    xhab = xh.rearrange("(b p) t -> p b t", p=128)
    xlab = xl.rearrange("(b p) t -> p b t", p=128)
    ysab = [yc.rearrange("(b p) t -> p b t", p=128) for yc in ys]

    NOB = 6   # out staging buffers (1 chunk each)
    NPB = 7   # psum accumulation banks (bank 6 doubles as warmup scratch)
    MAXC = max(CHUNKS)

    def xal(c):
        n = CHUNKS[c] + 2 * PAD
        return (n + 15) // 16 * 16

    with tile.TileContext(nc) as tc:
        with tc.tile_pool(name="wp", bufs=1) as wp, \
             tc.tile_pool(name="pp", bufs=1, space="PSUM") as pp:

            # write-once observer scratch: two columns per observer matmul
            obs_ps = pp.tile([128, 128], f32, name="obs_ps", tag="obs")
            pbufs = [pp.tile([128, SUB], f32, name=f"pt{j}", tag=f"pt{j}")
                     for j in range(NPB)]
            pdum = pbufs[NPB - 1]
            # x fully resident: one dedicated buffer per chunk, no reuse
            xhbufs = {c: wp.tile([128, 2, xal(c)], f8, name=f"xht{c}",
                              tag=f"xht{c}") for c in range(4, NCHUNK)}
            xlbufs = {c: wp.tile([128, 2, xal(c)], f8, name=f"xlt{c}",
                              tag=f"xlt{c}") for c in range(4, NCHUNK)}
            xpt = [wp.tile([128, 4 * xal(c)], f8, name=f"xpt{c}")
                   for c in range(4)]
            obufs = [wp.tile([128, 2, MAXC], mybir.dt.bfloat16,
                             name=f"ot{j}", tag=f"ot{j}")
                     for j in range(NOB)]
            # write-once gate scratch: one column per gate memset
            gs = wp.tile([128, 2 * NCHUNK + 8], f32, name="gs")
            pgs = wp.tile([128, NCHUNK + 8], f32, name="pgs")
            w0ht = wp.tile([128, K * 2 * 128], f8, name="w0ht")
            w0lt = wp.tile([128, K * 2 * 128], f8, name="w0lt")
            w1ct = wp.tile([128, 2 * K * 2 * 128], f8, name="w1ct")
            WR = K * 2 * 128
            whr = [w0ht[:, :].rearrange("p (k b m) -> p k b m", k=K, b=2),
                   w1ct[:, 0:WR].rearrange("p (k b m) -> p k b m", k=K, b=2)]
            wlr = [w0lt[:, :].rearrange("p (k b m) -> p k b m", k=K, b=2),
                   w1ct[:, WR:2 * WR].rearrange("p (k b m) -> p k b m",
                                                k=K, b=2)]
            bs = wp.tile([128, 2], f32, name="bs")
            bscratch = wp.tile([128, 2], f32, name="bscratch")
            wz = wp.tile([128, SUB], f8, name="wz")

            # --- PE warmup: keep PE busy from ~1.5us so the p-state ramp
            # finishes before the first real matmul (idle resets the ramp).
            nc.vector.memset(wz[:], 0.0)
            # t=T pad columns via DVE memset (no DMA, no observer;
            # consumers' DVE wait rides the same monotonic DVE sem).
            lastc = NCHUNK - 1
            le = CHUNKS[lastc] + PAD
            nc.vector.memset(xhbufs[lastc][:, :, le:le + PAD], 0.0)
            nc.vector.memset(xlbufs[lastc][:, :, le:le + PAD], 0.0)
            for _ in range(N_WARM):
                nc.tensor.matmul(pdum[:], wz[:, 0:128], wz[:], start=True,
                                 stop=True)

            def load_x(c, xt, xab, eng):
                """Issue chunk c's x DMA (SWDGE when eng is None).
                Returns (dma, observer_src_ap)."""
                n = CHUNKS[c]
                lo = CSTART[c] - PAD
                def start(out_ap, in_ap):
                    if eng is None:
                        return pool_dma(out_ap, in_ap)
                    d = eng.dma_start(out=out_ap, in_=in_ap)
                    return d
                if c == NCHUNK - 1:
                    d = start(xt[:, :, 0:n + PAD], xab[:, :, lo:T])
                else:
                    d = start(xt[:, :, 0:n + 2 * PAD],
                              xab[:, :, lo:lo + n + 2 * PAD])
                return (d, xt[:, 0, 0:128])

            # ---- Upfront DMA issue (sequencers run ahead of PE). ----
            # HWDGE descriptor-gen is one shared serial device (~630ns per
            # DMA), so the fill-critical early loads (xh0, xl0..xl3) go via
            # SWDGE on the idle Pool engine instead; the SP HWDGE ring leads
            # with the four weight quarters.
            pool_dmas = []    # all Pool (SWDGE) DMAs in issue order
            hw_dmas = []      # all HWDGE DMAs in issue order

            def pool_dma(out_ap, in_ap):
                pool_dmas.append(nc.gpsimd.dma_start(out=out_ap, in_=in_ap))
                return pool_dmas[-1]

            # Everything HWDGE rides the SP ring, in priority order - the
            # shared HWDGE gen device then serves exactly this sequence.
            # Pool (SWDGE) carries xp0 and, after a filler memset that
            # delays their transfer-queue readiness below xp1's, xp2/xp3.
            d_xp = [None] * 4
            d_xp[0] = pool_dma(xpt[0][:], xps[0][:])
            pfill = wp.tile([128, 1440], f8, name="pfill")
            nc.gpsimd.memset(pfill[:], 0.0)
            d_xp[2] = pool_dma(xpt[2][:], xps[2][:])
            d_xp[3] = pool_dma(xpt[3][:], xps[3][:])
            d_w0h = nc.sync.dma_start(out=w0ht[:], in_=w0h[:])
            d_w0l = nc.sync.dma_start(out=w0lt[:], in_=w0l[:])
            d_b = nc.sync.dma_start(out=bs[:], in_=bias[:])
            d_w1c = nc.sync.dma_start(out=w1ct[:], in_=w1c[:])
            d_xp[1] = nc.sync.dma_start(out=xpt[1][:], in_=xps[1][:])
            hw_dmas.extend([d_w0h, d_w0l, d_b, d_w1c, d_xp[1]])
            d_w = [[d_w0h, d_w1c], [d_w0l, d_w1c]]   # [hl][co]
            h_recs, l_recs = {}, {}
            for c in range(4, NCHUNK):
                h_recs[c] = load_x(c, xhbufs[c], xhab, nc.sync)
                hw_dmas.append(h_recs[c][0])
                l_recs[c] = load_x(c, xlbufs[c], xlab, nc.sync)
                hw_dmas.append(l_recs[c][0])
            # packed-x views for chunks 0..3: [xh | xl], each [128, 2, XAL]
            xh_view, xl_view = {}, {}
            for c in range(4):
                a = xal(c)
                xh_view[c] = xpt[c][:, 0:2 * a].rearrange(
                    "p (b t) -> p b t", b=2)
                xl_view[c] = xpt[c][:, 2 * a:4 * a].rearrange(
                    "p (b t) -> p b t", b=2)

            # DVE observes the bias lane via a write-once copy
            obs_b = nc.vector.tensor_copy(bscratch[:], bs[:])

            n_obs = [0]

            def pe_observe(src_ap, dma_inst):
                """1-column matmul whose only wait is `dma_inst`'s lane."""
                n = src_ap.shape[-1]
                m = min(128, n)
                oc = 2 * n_obs[0]
                n_obs[0] += 1
                mm = nc.tensor.matmul(obs_ps[0:m, oc:oc + 2], src_ap[:, 0:m],
                                      src_ap[:, 0:2], start=True, stop=True)
                add_dep_helper(mm.ins, dma_inst.ins, sync=False,
                               reason="obs-order")
                return mm

            n_gate = [0]

            def dve_gate(dep_inst):
                """Write-once DVE memset whose only wait is dep's proc tick."""
                gc = n_gate[0]
                n_gate[0] += 1
                ms = nc.vector.memset(gs[:, gc:gc + 1], 0.0)
                add_dep_helper(ms.ins, dep_inst.ins, sync=True,
                               reason="dve-gate")
                return ms

            n_pgate = [0]

            def pool_gate(dep_inst):
                """Write-once Pool memset whose only wait is dep's tick."""
                gc = n_pgate[0]
                n_pgate[0] += 1
                ms = nc.gpsimd.memset(pgs[:, gc:gc + 1], 0.0)
                add_dep_helper(ms.ins, dep_inst.ins, sync=True,
                               reason="pool-gate")
                return ms

            out_dmas = []
            chunk_od = {}     # chunk -> its (last) store
            chunk_ev = {}     # chunk -> its last evict
            last_mm = None
            last_evict = None
            pi = 0            # psum bank rotation
            w_obs_done = {}   # (hl, co) -> observed?

            def issue_store(out_ap, in_ap, via_hwdge=False):
                if via_hwdge:
                    # end-of-kernel store on the idle SP HWDGE ring; a
                    # sequencer nop absorbs the HWDGE-lane recycle wait
                    i = len(hw_dmas)
                    if i >= 8:
                        nop = nc.sync.nop()
                        add_dep_helper(nop.ins, hw_dmas[i - 8].ins,
                                       sync=True, reason="hw-lane-gate")
                    od = nc.sync.dma_start(out=out_ap, in_=in_ap)
                    hw_dmas.append(od)
                else:
                    i = len(pool_dmas)
                    if i >= 8:
                        # absorb the DMASW lane-recycle wait onto Pool's clock
                        pool_gate(pool_dmas[i - 8])
                    od = nc.gpsimd.dma_start(out=out_ap, in_=in_ap)
                    pool_dmas.append(od)
                out_dmas.append(od)
                return od

            for c in range(NCHUNK):
                ncols = CHUNKS[c]
                # PE-inline observers: right before the first matmul that
                # reads the tiles (PE is in-order).
                if c < 4:
                    hobs = [pe_observe(xh_view[c][:, 0, 0:128], d_xp[c])]
                    lobs_pending = []
                else:
                    hobs = [pe_observe(h_recs[c][1], h_recs[c][0])]
                    lobs_pending = [l_recs[c]]

                ot = obufs[c % NOB]
                evict_gates = [obs_b]
                if c >= NOB:
                    # pre-lift the recycled out buffer's history (old evict
                    # WAW + old store WAR) onto DVE's observed clock: one
                    # 1-wait gate per dep
                    evict_gates.append(dve_gate(chunk_ev[c - NOB]))
                    evict_gates.append(dve_gate(chunk_od[c - NOB]))

                first_evict_of_chunk = True
                if c < 4:
                    xht, xlt = xh_view[c], xl_view[c]
                else:
                    xht, xlt = xhbufs[c], xlbufs[c]
                nts = ncols // SUB
                for co in range(2):
                    for ts in range(nts):
                        pt = pbufs[pi % NPB]
                        pi += 1
                        first = True
                        # pass order: wh@xh x5, wh@xl x5, wl@xh x5
                        for wr, xtile in ((whr[co], xht), (whr[co], xlt),
                                          (wlr[co], xht)):
                            if xtile is xlt and lobs_pending:
                                lobs = [pe_observe(r[1], r[0]) for r in lobs_pending]
                                lobs_pending = []
                            else:
                                lobs = []
                            hl = 1 if wr is wlr[co] else 0
                            if not w_obs_done.get((hl, co)):
                                lobs.append(pe_observe(wr[:, 0, 0],
                                                       d_w[hl][co]))
                                w_obs_done[(hl, co)] = True
                            for k in range(K):
                                off = ts * SUB + k
                                mm = nc.tensor.matmul(
                                    pt[:],
                                    wr[:, k],
                                    xtile[:, :, off:off + SUB],
                                    start=first,
                                    stop=(hl == 1 and k == K - 1),
                                    perf_mode=mybir.MatmulPerfMode.DoubleRow,
                                )
                                if k == 0:
                                    gates = lobs + (hobs if first else [])
                                    for ob in gates:
                                        add_dep_helper(
                                            mm.ins, ob.ins, sync=False,
                                            reason="order-after-observe")
                                first = False
                                last_mm = mm
                        ooff = ts * SUB
                        ev = nc.vector.tensor_scalar(
                            out=ot[:, co, ooff:ooff + SUB],
                            in0=pt[:],
                            scalar1=float(OSCALE),
                            scalar2=bs[:, co:co + 1],
                            op0=mybir.AluOpType.mult,
                            op1=mybir.AluOpType.add,
                        )
                        if first_evict_of_chunk:
                            for g in evict_gates:
                                add_dep_helper(ev.ins, g.ins, sync=False,
                                               reason="order-after-gate")
                            first_evict_of_chunk = False
                        last_evict = ev
                        chunk_ev[c] = ev

                        if c == NCHUNK - 1:
                            # final chunk: per-co stores so the exit drain
                            # waits only on a 512-col store
                            chunk_od[c] = issue_store(
                                ysab[c][:, co, 0:ncols],
                                ot[:, co, 0:ncols])

                if c != NCHUNK - 1:
                    # streaming store: the whole chunk as one SWDGE DMA,
                    # fired right after its last eviction
                    chunk_od[c] = issue_store(
                        ysab[c][:], ot[:, :, 0:ncols])

            # Tail flush: cover every proc with 1-dep sync nops so the
            # final drain carries at most one wait.
            tail_deps = [h_recs[c][0] for c in range(NCHUNK - 8, NCHUNK)] + \
                [l_recs[c][0] for c in range(NCHUNK - 8, NCHUNK)] + \
                out_dmas + [last_mm, last_evict]
            for dep in tail_deps:
                nop = nc.sync.nop()
                add_dep_helper(nop.ins, dep.ins, sync=True, reason="tailflush")

    return nc


def check_waits(nc):
    """Return instructions carrying more than one sync wait (walrus limit)."""
    bad = []
    for f in nc.m.functions:
        for bb in f.blocks:
            for inst in bb.instructions:
                si = inst.sync_info
                nw = len(si.on_wait) if si and si.on_wait else 0
                if nw > 1:
                    bad.append((inst.name, type(inst).__name__, nw,
                                [w.ant_name for w in si.on_wait]))
    return bad


def _q8(a):
    return np.asarray(a, dtype=np.float32).astype(E4)


def _pack_weights(conv_w, conv_b, lora_A, lora_B):
    w_eff = conv_w.astype(np.float32) + (
        SCALING * np.einsum(
            "or,rik->oik", lora_B.astype(np.float64),
            lora_A.astype(np.float64).reshape(RANK, CI, K))
    ).astype(np.float32)
    wp = w_eff * np.float32(WS)
    wh = _q8(wp)
    wl = _q8(wp - wh.astype(np.float32))
    # stationary pack: [ki, k*256 + b*128 + m] = w_hl[co*128+m, b*128+ki, k]
    wp4 = {}
    for hl, arr in (("h", wh), ("l", wl)):
        for co in range(2):
            a = arr.reshape(2, 128, 2, 128, K)[co]  # [m, b, ki, k]
            a = a.transpose(2, 3, 1, 0)             # [ki, k, b, m]
            wp4[(hl, co)] = np.ascontiguousarray(a.reshape(128, K * 2 * 128))
    packs = {"w0h": wp4[("h", 0)], "w0l": wp4[("l", 0)],
             "w1c": np.concatenate([wp4[("h", 1)], wp4[("l", 1)]], axis=1)}
    bias = np.ascontiguousarray(
        conv_b.astype(np.float32).reshape(2, 128).T)  # [128, 2]
    return packs, bias


def _pack_x_chunks(xh, xl):
    """Per-core packed [xh | xl] fill tensors for chunks 0..3.
    Layout per partition: [b, XAL] halo windows (pads zeroed)."""
    CH = [512, 512, 1024, 1024]
    CS = [0, 512, 1024, 2048]
    out = []
    for c in range(4):
        a = (CH[c] + 2 * PAD + 15) // 16 * 16
        buf = np.zeros((2, 2, 128, a), dtype=E4)   # [hl, b, p, t]
        lo = CS[c] - PAD
        s0, d0 = max(0, lo), max(0, -lo)
        e0 = CS[c] + CH[c] + PAD
        for i, arr in enumerate((xh, xl)):
            w = arr.reshape(2, 128, T)[:, :, s0:e0]
            buf[i, :, :, d0:d0 + e0 - s0] = w
        out.append(np.ascontiguousarray(
            buf.transpose(2, 0, 1, 3).reshape(128, 4 * a)))
    return out


_CACHED_NC = None


def kernel(x, conv_w, conv_b, lora_A, lora_B, _trace=False):
    global _CACHED_NC
    x = np.asarray(x, dtype=np.float32)
    packs, bias = _pack_weights(np.asarray(conv_w), np.asarray(conv_b),
                                np.asarray(lora_A), np.asarray(lora_B))
    xs = x * np.float32(XS)
    xh = _q8(xs)
    xlo = _q8(xs - xh.astype(np.float32))

    if _CACHED_NC is None:
        _CACHED_NC = _build_nc()
        bad = check_waits(_CACHED_NC)
        assert not bad, f"sync-wait violations: {bad[:5]}"
    nc = _CACHED_NC

    in_maps = []
    for i in range(N_CORES):
        m = dict(packs, xh=xh[i], xl=xlo[i], bias=bias)
        for c, xp in enumerate(_pack_x_chunks(xh[i], xlo[i])):
            m[f"x{c}p"] = xp
        in_maps.append(m)
    res = bass_utils.run_bass_kernel_spmd(
        nc, in_maps, core_ids=list(range(N_CORES)), trace=_trace)
    out = np.stack(
        [np.concatenate([res.results[i][f"y{c}"] for c in range(NCHUNK)],
                        axis=1)
         for i in range(N_CORES)], axis=0).astype(np.float32)
    if _trace:
        kernel._last_exec_time_ns = res.exec_time_ns
        kernel._last_results = res
    return out


if __name__ == "__main__":
    nc = _build_nc()
    bad = check_waits(nc)
    print("violations:", bad[:10])
    n_inst = sum(len(bb.instructions) for f in nc.m.functions for bb in f.blocks)
    print("instructions:", n_inst)
